# revision 4
# baseline (speedup 1.0000x reference)
"""GCN layer relu((A_hat @ x) @ W + b) on 8 TRN2 NeuronCores (Bass/Tile).

Strategy (dst-sharded SPMD, one program on 8 cores):
  - nodes (rows of x / output) split into 8 contiguous shards of 12500;
    edges partitioned by destination so the one-hot-matmul scatter-add is
    device-local. The small 128x128 weight is replicated.
  - x is processed in 4 source chunks of 25088 rows; each chunk is DMA'd
    sequentially (large, fast transfer) into SBUF in a token layout
    (token i -> partition i%128, free offset (i//128)*256B), and per-edge
    source rows are then expanded with SWDGE dma_gather reading from SBUF
    (measured at the same ~9-10 ns/token descriptor-rate floor as an HBM
    source, but it removes the HBM random-row traffic and lets the
    sequential chunk loads run at line rate).
  - edges regioned by (chunk, 512-wide dst region), padded to 128-slot
    blocks with the max count across cores so one SPMD program fits all
    8 cores; pad slots use spread dummy indices and are killed by norm=0.
    RW=512 minimizes total padded slots (the gather is the wall and its
    cost is per-token).
  - per gathered block: TensorE transpose-back via identity (4 transposes
    batched into one PSUM bank, drained by a single ScalarE copy), then
    DVE builds a scaled one-hot st[slot, d512] = (cmp==rel)*nrm (fp16 cmp:
    integers up to 511 must be exact, bf16 only covers 0..256), and
    TensorE accumulates psum[feat, d512] += g_block.T @ st.
  - agg kept in [feat, dst] bf16; the self-loop term dinv^2[d]*x[d] is
    host-prescaled and streamed per region at first-chunk flush.
  - phase 2 per 128-dst window after a region's last flush: matmul W
    straight from the agg slice, DVE bias+relu, DMA out.
  - scatter/matmul math in bf16 with fp32 PSUM accumulation
    (rel err ~4e-3, well within 2e-2).
"""
import math

import numpy as np
import ml_dtypes

import concourse.bacc as bacc
import concourse.mybir as mybir
import concourse.tile as tile
from concourse import bass_utils

P = 128
FEAT = 128
N_NODES = 100000
NCORES = 8
WINDOW = 128
RW = 512          # dst span per gather region
S = RW // WINDOW
CHUNK_ROWS = 25088            # 196 ranks of 128 tokens; int16-safe indices
CALL_MAX_BLOCKS = 32

NPC = N_NODES // NCORES                   # 12500 dst nodes per core
NW = math.ceil(NPC / WINDOW)              # 98 output windows per core
NR = math.ceil(NPC / RW)                  # gather regions per chunk
DPAD = NW * WINDOW                        # 12544 output dst rows per core
APAD = NR * RW                            # agg dst columns per core
NCHUNK = math.ceil(N_NODES / CHUNK_ROWS)  # 4 src chunks
RANKS = CHUNK_ROWS // P                   # 196 token ranks per chunk


def _bf16(a):
    return a.astype(ml_dtypes.bfloat16)


def _host_prep(x, edge_index, edge_weight, W, b):
    n = N_NODES
    src = np.asarray(edge_index[0], dtype=np.int64)
    dst = np.asarray(edge_index[1], dtype=np.int64)
    ew = np.asarray(edge_weight, dtype=np.float32)

    deg = np.bincount(dst, weights=ew.astype(np.float64), minlength=n)
    deg = (deg + 1.0).astype(np.float32)  # + self-loop weight
    dinv = np.where(deg > 0, 1.0 / np.sqrt(deg), 0.0).astype(np.float32)
    norm = (dinv[src] * ew * dinv[dst]).astype(np.float32)
    dinv2 = (dinv * dinv).astype(np.float32)

    core = dst // NPC
    dst_local = dst - core * NPC
    w_id = dst_local // RW
    rel = (dst_local - w_id * RW).astype(np.float32)
    c_id = src // CHUNK_ROWS
    idx_local = (src - c_id * CHUNK_ROWS).astype(np.int16)

    flat = (core * NCHUNK + c_id) * NR + w_id
    counts = np.bincount(flat, minlength=NCORES * NCHUNK * NR).reshape(
        NCORES, NCHUNK, NR)
    B = np.ceil(counts.max(axis=0) / P).astype(np.int64)

    nb_total = int(B.sum())
    slots_total = nb_total * P

    block_base = np.zeros((NCHUNK, NR), dtype=np.int64)
    regions = []
    acc = 0
    for c in range(NCHUNK):
        for w in range(NR):
            block_base[c, w] = acc
            regions.append((c, w, acc, int(B[c, w])))
            acc += int(B[c, w])

    calls = []
    for c in range(NCHUNK):
        b0 = int(block_base[c, 0])
        b1 = int(block_base[c + 1, 0]) if c + 1 < NCHUNK else nb_total
        k = b0
        while k < b1:
            nblk = min(CALL_MAX_BLOCKS, b1 - k)
            calls.append((c, k, nblk))
            k += nblk

    meta = dict(regions=regions, calls=calls, nb_total=nb_total,
                slots_total=slots_total, B=B)

    order_all = np.lexsort((w_id, c_id, core))
    core_sorted = core[order_all]
    core_starts = np.searchsorted(core_sorted, np.arange(NCORES + 1))

    x32 = np.ascontiguousarray(np.asarray(x, dtype=np.float32))
    # token-layout bf16 copy of x: chunk c, rank r, partition p, feat f
    xpad = np.zeros((NCHUNK * CHUNK_ROWS, FEAT), dtype=np.float32)
    xpad[:n] = x32
    xgr = _bf16(
        np.ascontiguousarray(
            xpad.reshape(NCHUNK, RANKS, P, FEAT)
            .transpose(2, 0, 1, 3)
            .reshape(P, NCHUNK * RANKS * FEAT)))
    W16 = _bf16(np.ascontiguousarray(np.asarray(W, dtype=np.float32)))
    b32 = np.asarray(b, dtype=np.float32)
    btile = np.tile(b32[None, :], (P, 1)).astype(np.float32)
    cmp_t = np.tile(np.arange(RW, dtype=np.float32)[None, :],
                    (P, 1)).astype(np.float16)
    ident = _bf16(np.eye(P, dtype=np.float32))

    in_maps = []
    for m in range(NCORES):
        sel = order_all[core_starts[m]:core_starts[m + 1]]
        midx, mrel, mnorm = idx_local[sel], rel[sel], norm[sel]

        rng = np.random.default_rng(12345 + m)
        idx16 = rng.integers(0, CHUNK_ROWS, slots_total).astype(np.int16)
        relq = np.full(slots_total, -1.0, dtype=np.float32)
        nrm = np.zeros(slots_total, dtype=np.float32)
        pos = 0
        for (c, w, blk0, nblk) in regions:
            cnt = int(counts[m, c, w])
            s0 = blk0 * P
            idx16[s0:s0 + cnt] = midx[pos:pos + cnt]
            relq[s0:s0 + cnt] = mrel[pos:pos + cnt]
            nrm[s0:s0 + cnt] = mnorm[pos:pos + cnt]
            pos += cnt
        assert pos == len(sel)

        idx_tile = np.zeros((P, slots_total // 16), dtype=np.int16)
        for (c, blk0, nblk) in calls:
            s0, s1 = blk0 * P, (blk0 + nblk) * P
            seg = idx16[s0:s1].reshape(-1, 16).T
            idx_tile[:, s0 // 16:s1 // 16] = np.tile(seg, (8, 1))

        # self-loop term pre-scaled on host, [feat, APAD dst] layout
        xs = np.zeros((APAD, FEAT), dtype=np.float32)
        xs[:NPC] = x32[m * NPC:(m + 1) * NPC]
        xs[:NPC] *= dinv2[m * NPC:(m + 1) * NPC, None]
        xslT = np.ascontiguousarray(xs.T)

        in_maps.append({
            "xgr": xgr,
            "idx": idx_tile,
            "rel": relq.reshape(nb_total, P).T.copy(),
            "nrm": nrm.reshape(nb_total, P).T.copy(),
            "cmp": cmp_t,
            "xslT": _bf16(xslT),
            "Wt": W16,
            "btile": btile,
            "ident": ident,
        })
    return meta, in_maps


def _build_kernel(meta, repeat=1, mode="full"):
    nb_total = meta["nb_total"]
    slots_total = meta["slots_total"]
    regions = meta["regions"]
    calls = meta["calls"]
    gdt = mybir.dt.bfloat16
    do_gather = mode in ("full", "gather_only")
    do_compute = mode in ("full", "compute_only")

    nc = bacc.Bacc("TRN2", target_bir_lowering=False, debug=False,
                   num_devices=NCORES)
    xgr = nc.dram_tensor("xgr", [P, NCHUNK * RANKS * FEAT], gdt,
                         kind="ExternalInput")
    idx = nc.dram_tensor("idx", [P, slots_total // 16], mybir.dt.int16,
                         kind="ExternalInput")
    rel = nc.dram_tensor("rel", [P, nb_total], mybir.dt.float32,
                         kind="ExternalInput")
    nrm = nc.dram_tensor("nrm", [P, nb_total], mybir.dt.float32,
                         kind="ExternalInput")
    cmp_d = nc.dram_tensor("cmp", [P, RW], mybir.dt.float16,
                           kind="ExternalInput")
    xslT = nc.dram_tensor("xslT", [FEAT, APAD], gdt, kind="ExternalInput")
    Wt = nc.dram_tensor("Wt", [FEAT, FEAT], gdt, kind="ExternalInput")
    btile = nc.dram_tensor("btile", [P, FEAT], mybir.dt.float32,
                           kind="ExternalInput")
    ident = nc.dram_tensor("ident", [P, P], gdt, kind="ExternalInput")
    out = nc.dram_tensor("out", [DPAD, FEAT], mybir.dt.float32,
                         kind="ExternalOutput")

    Bm = meta["B"]
    first_c = {}
    last_c = {}
    for w in range(NR):
        cs = [c for c in range(NCHUNK) if Bm[c, w] > 0]
        first_c[w] = cs[0] if cs else None
        last_c[w] = cs[-1] if cs else None

    with tile.TileContext(nc) as tc:
        with (
            tc.tile_pool(name="const", bufs=1) as constp,
            tc.tile_pool(name="agg", bufs=1) as aggp,
            tc.tile_pool(name="chunk", bufs=2) as chunkp,
            tc.tile_pool(name="gbuf", bufs=2) as gbufp,
            tc.tile_pool(name="gsc", bufs=2) as gscp,
            tc.tile_pool(name="sel", bufs=6) as selp,
            tc.tile_pool(name="xsl", bufs=3) as xslp,
            tc.tile_pool(name="ps1", bufs=2, space="PSUM") as ps1p,
            tc.tile_pool(name="tpg", bufs=3, space="PSUM") as tpgp,
            tc.tile_pool(name="ps2", bufs=2, space="PSUM") as ps2p,
            tc.tile_pool(name="outst", bufs=3) as outp,
        ):
            idx_sb = constp.tile([P, slots_total // 16], mybir.dt.int16)
            rel_sb = constp.tile([P, nb_total], mybir.dt.float32)
            nrm_sb = constp.tile([P, nb_total], mybir.dt.float32)
            cmp_sb = constp.tile([P, RW], mybir.dt.float16)
            W_sb = constp.tile([FEAT, FEAT], gdt)
            b_sb = constp.tile([P, FEAT], mybir.dt.float32)
            id_sb = constp.tile([P, P], gdt)
            agg = aggp.tile([FEAT, APAD], gdt)

            for _rep in range(repeat):
                nc.sync.dma_start(out=idx_sb[:], in_=idx[:])
                nc.sync.dma_start(out=rel_sb[:], in_=rel[:])
                nc.sync.dma_start(out=nrm_sb[:], in_=nrm[:])
                nc.sync.dma_start(out=cmp_sb[:], in_=cmp_d[:])
                nc.sync.dma_start(out=W_sb[:], in_=Wt[:])
                nc.sync.dma_start(out=b_sb[:], in_=btile[:])
                nc.sync.dma_start(out=id_sb[:], in_=ident[:])

                chunk_sb = {}

                def load_chunk(c):
                    t = chunkp.tile([P, RANKS * FEAT], gdt, tag="chunk")
                    nc.sync.dma_start(
                        out=t[:],
                        in_=xgr[:, c * RANKS * FEAT:(c + 1) * RANKS * FEAT])
                    chunk_sb[c] = t

                load_chunk(0)
                if NCHUNK > 1:
                    load_chunk(1)

                gtiles = {}
                issued = set()
                call_of_block = {}
                for ci, (c, blk0, nblk) in enumerate(calls):
                    for bb in range(blk0, blk0 + nblk):
                        call_of_block[bb] = ci

                def gather_call(ci):
                    c, blk0, nblk = calls[ci]
                    nidx = nblk * P
                    g = gbufp.tile([P, 1, CALL_MAX_BLOCKS * P], gdt, tag="g")
                    if not do_gather:
                        nc.vector.memset(g[:, :, :nidx], 0.25)
                    if do_gather:
                        nc.gpsimd.dma_gather(
                            g[:, :, :nidx],
                            chunk_sb[c][:],
                            idx_sb[:, blk0 * 8:(blk0 + nblk) * 8],
                            nidx, nidx, FEAT,
                            transpose=True, single_packet=False,
                            sbuf_tokens_per_rank=P,
                            sbuf_free_dim_per_rank=FEAT * 2,
                        )
                    if do_compute:
                        gsc_call = gscp.tile([P, CALL_MAX_BLOCKS, FEAT], gdt,
                                             tag="gsc")
                        TB = 4
                        for b0 in range(0, nblk, TB):
                            nb2 = min(TB, nblk - b0)
                            tpg = tpgp.tile([P, TB, P], gdt, tag="tpg")
                            for k in range(nb2):
                                bb = b0 + k
                                nc.tensor.transpose(
                                    tpg[:, k, :],
                                    g[:, 0, bb * P:(bb + 1) * P], id_sb[:])
                            nc.scalar.activation(
                                gsc_call[:, b0:b0 + nb2, :],
                                tpg[:, :nb2, :],
                                mybir.ActivationFunctionType.Copy)
                        gtiles[ci] = (gsc_call, blk0, nblk)
                    else:
                        gtiles[ci] = (None, blk0, nblk)

                def phase2(w):
                    ps2 = ps2p.tile([P, FEAT], mybir.dt.float32, tag="ps2")
                    nc.tensor.matmul(out=ps2[:],
                                     lhsT=agg[:, w * P:(w + 1) * P],
                                     rhs=W_sb[:], start=True, stop=True)
                    ot = outp.tile([P, FEAT], mybir.dt.float32, tag="ot")
                    nc.vector.tensor_add(out=ot[:], in0=ps2[:], in1=b_sb[:])
                    nc.vector.tensor_scalar_max(ot[:], ot[:], 0.0)
                    nc.sync.dma_start(out=out[w * P:(w + 1) * P, :], in_=ot[:])

                for (c, r2, blk0, nblk) in regions:
                    d0 = r2 * RW
                    wsl = agg[:, d0:d0 + RW]
                    if not do_compute:
                        for j, gb in enumerate(range(blk0, blk0 + nblk)):
                            ci = call_of_block[gb]
                            if ci not in issued:
                                gather_call(ci)
                                issued.add(ci)
                        if c + 2 < NCHUNK and r2 == NR - 1:
                            load_chunk(c + 2)
                        continue
                    if nblk == 0:
                        if first_c[r2] is None and c == 0:
                            xsl_t = xslp.tile([FEAT, RW], gdt, tag="xsl")
                            nc.sync.dma_start(out=xsl_t[:],
                                              in_=xslT[:, d0:d0 + RW])
                            nc.vector.tensor_copy(out=wsl, in_=xsl_t[:])
                            for s in range(S):
                                if r2 * S + s < NW:
                                    phase2(r2 * S + s)
                        continue
                    if c == first_c[r2]:
                        xsl_t = xslp.tile([FEAT, RW], gdt, tag="xsl")
                        nc.sync.dma_start(out=xsl_t[:],
                                          in_=xslT[:, d0:d0 + RW])
                    ps = ps1p.tile([P, RW], mybir.dt.float32, tag="ps1")
                    for j, gb in enumerate(range(blk0, blk0 + nblk)):
                        ci = call_of_block[gb]
                        if ci not in issued:
                            gather_call(ci)
                            issued.add(ci)
                        g, cblk0, cnblk = gtiles[ci]
                        col = gb - cblk0
                        st = selp.tile([P, RW], gdt, tag="sel")
                        nc.vector.tensor_scalar(
                            out=st[:], in0=cmp_sb[:],
                            scalar1=rel_sb[:, gb:gb + 1],
                            scalar2=nrm_sb[:, gb:gb + 1],
                            op0=mybir.AluOpType.is_equal,
                            op1=mybir.AluOpType.mult,
                        )
                        nc.tensor.matmul(
                            out=ps[:], lhsT=g[:, col, :], rhs=st[:],
                            start=(j == 0), stop=(j == nblk - 1),
                        )
                    if c == first_c[r2]:
                        nc.vector.tensor_tensor(
                            out=wsl, in0=ps[:], in1=xsl_t[:],
                            op=mybir.AluOpType.add)
                    else:
                        nc.vector.tensor_add(out=wsl, in0=wsl, in1=ps[:])
                    if c == last_c[r2]:
                        for s in range(S):
                            if r2 * S + s < NW:
                                phase2(r2 * S + s)
                    if c + 2 < NCHUNK and r2 == NR - 1:
                        load_chunk(c + 2)
    nc.compile()
    return nc


def kernel(x, edge_index, edge_weight, W, b):
    assert x.shape == (N_NODES, FEAT)
    meta, in_maps = _host_prep(x, edge_index, edge_weight, W, b)
    nc = _build_kernel(meta)
    res = bass_utils.run_bass_kernel_spmd(
        nc, in_maps, core_ids=list(range(NCORES)), trace=False)
    outs = [res.results[m]["out"][:NPC] for m in range(NCORES)]
    return np.ascontiguousarray(np.concatenate(outs, axis=0))


# revision 9
# speedup vs baseline: 1.0162x; 1.0162x over previous
"""GCN layer relu((A_hat @ x) @ W + b) on 8 TRN2 NeuronCores (Bass/Tile).

Strategy (dst-sharded SPMD, one program on 8 cores):
  - nodes (rows of x / output) split into 8 contiguous shards of 12500;
    edges partitioned by destination so the one-hot-matmul scatter-add is
    device-local. The small 128x128 weight is replicated.
  - x is processed in 4 source chunks of 25088 rows; each chunk is DMA'd
    sequentially (large, fast transfer) into SBUF in a token layout
    (token i -> partition i%128, free offset (i//128)*256B), and per-edge
    source rows are then expanded with SWDGE dma_gather reading from SBUF
    (measured at the same ~9-10 ns/token descriptor-rate floor as an HBM
    source, but it removes the HBM random-row traffic and lets the
    sequential chunk loads run at line rate).
  - edges regioned by (chunk, 512-wide dst region), padded to 128-slot
    blocks with the max count across cores so one SPMD program fits all
    8 cores; pad slots use spread dummy indices and are killed by norm=0.
    RW=512 minimizes total padded slots (the gather is the wall and its
    cost is per-token). Dst nodes are greedily permuted into regions on
    host so per-(chunk, region) counts balance across cores (shrinks the
    max-over-cores padding ~1.3%); output rows are un-permuted on host.
  - per gathered block: TensorE transpose-back via identity (4 transposes
    batched into one PSUM bank, drained by a single ScalarE copy), then
    DVE builds a scaled one-hot st[slot, d512] = (cmp==rel)*nrm (fp16 cmp:
    integers up to 511 must be exact, bf16 only covers 0..256), and
    TensorE accumulates psum[feat, d512] += g_block.T @ st.
  - agg kept in [feat, dst] bf16; the self-loop term dinv^2[d]*x[d] is
    host-prescaled and streamed per region at first-chunk flush.
  - phase 2 per 128-dst window after a region's last flush: matmul W
    straight from the agg slice, DVE bias+relu, DMA out.
  - scatter/matmul math in bf16 with fp32 PSUM accumulation
    (rel err ~4e-3, well within 2e-2).
"""
import math

import numpy as np
import ml_dtypes

import concourse.bacc as bacc
import concourse.mybir as mybir
import concourse.tile as tile
from concourse import bass_utils

P = 128
FEAT = 128
N_NODES = 100000
NCORES = 8
WINDOW = 128
RW = 512          # dst span per gather region
S = RW // WINDOW
CHUNK_ROWS = 25088            # 196 ranks of 128 tokens; int16-safe indices
CALL_MAX_BLOCKS = 32

NPC = N_NODES // NCORES                   # 12500 dst nodes per core
NW = math.ceil(NPC / WINDOW)              # 98 output windows per core
NR = math.ceil(NPC / RW)                  # gather regions per chunk
DPAD = NW * WINDOW                        # 12544 output dst rows per core
APAD = NR * RW                            # agg dst columns per core
NCHUNK = math.ceil(N_NODES / CHUNK_ROWS)  # 4 src chunks
RANKS = CHUNK_ROWS // P                   # 196 token ranks per chunk


def _bf16(a):
    return a.astype(ml_dtypes.bfloat16)


def _host_prep(x, edge_index, edge_weight, W, b):
    n = N_NODES
    src = np.asarray(edge_index[0], dtype=np.int64)
    dst = np.asarray(edge_index[1], dtype=np.int64)
    ew = np.asarray(edge_weight, dtype=np.float32)

    deg = np.bincount(dst, weights=ew.astype(np.float64), minlength=n)
    deg = (deg + 1.0).astype(np.float32)  # + self-loop weight
    dinv = np.where(deg > 0, 1.0 / np.sqrt(deg), 0.0).astype(np.float32)
    norm = (dinv[src] * ew * dinv[dst]).astype(np.float32)
    dinv2 = (dinv * dinv).astype(np.float32)

    core = dst // NPC
    dst_local = dst - core * NPC
    c_id = src // CHUNK_ROWS
    idx_local = (src - c_id * CHUNK_ROWS).astype(np.int16)

    # Balance dst nodes across regions so per-(chunk, region) edge counts
    # even out across cores: the SPMD program pads every region block count
    # to the max over the 8 cores, so imbalance turns directly into padded
    # gather tokens (the per-token gather floor is the wall). Greedy: place
    # nodes (largest degree first) into the region minimizing the peak
    # per-chunk count. Output rows come out permuted; unscrambled on host.
    nodecnt = np.bincount(dst * NCHUNK + c_id,
                          minlength=n * NCHUNK).reshape(n, NCHUNK)
    caps = np.full(NR, RW, dtype=np.int64)
    caps[NR - 1] = DPAD - (NR - 1) * RW
    newloc = np.empty((NCORES, NPC), dtype=np.int64)
    for m in range(NCORES):
        ncm = nodecnt[m * NPC:(m + 1) * NPC].astype(np.float64)
        order = np.argsort(-ncm.sum(axis=1), kind="stable")
        rc = np.zeros((NR, NCHUNK))
        rfill = np.zeros(NR, dtype=np.int64)
        for v in order:
            cand = rc + ncm[v]
            score = cand.max(axis=1) + 1e-4 * cand.sum(axis=1)
            score[rfill >= caps] = np.inf
            r = int(np.argmin(score))
            newloc[m, v] = r * RW + rfill[r]
            rc[r] += ncm[v]
            rfill[r] += 1

    nl = newloc[core, dst_local]
    w_id = nl // RW
    rel = (nl - w_id * RW).astype(np.float32)

    flat = (core * NCHUNK + c_id) * NR + w_id
    counts = np.bincount(flat, minlength=NCORES * NCHUNK * NR).reshape(
        NCORES, NCHUNK, NR)
    B = np.ceil(counts.max(axis=0) / P).astype(np.int64)

    nb_total = int(B.sum())
    slots_total = nb_total * P

    block_base = np.zeros((NCHUNK, NR), dtype=np.int64)
    regions = []
    acc = 0
    for c in range(NCHUNK):
        for w in range(NR):
            block_base[c, w] = acc
            regions.append((c, w, acc, int(B[c, w])))
            acc += int(B[c, w])

    calls = []
    for c in range(NCHUNK):
        b0 = int(block_base[c, 0])
        b1 = int(block_base[c + 1, 0]) if c + 1 < NCHUNK else nb_total
        k = b0
        while k < b1:
            nblk = min(CALL_MAX_BLOCKS, b1 - k)
            calls.append((c, k, nblk))
            k += nblk

    meta = dict(regions=regions, calls=calls, nb_total=nb_total,
                slots_total=slots_total, B=B, newloc=newloc)

    order_all = np.lexsort((w_id, c_id, core))
    core_sorted = core[order_all]
    core_starts = np.searchsorted(core_sorted, np.arange(NCORES + 1))

    x32 = np.ascontiguousarray(np.asarray(x, dtype=np.float32))
    # token-layout bf16 copy of x: chunk c, rank r, partition p, feat f
    xpad = np.zeros((NCHUNK * CHUNK_ROWS, FEAT), dtype=np.float32)
    xpad[:n] = x32
    xgr = _bf16(
        np.ascontiguousarray(
            xpad.reshape(NCHUNK, RANKS, P, FEAT)
            .transpose(2, 0, 1, 3)
            .reshape(P, NCHUNK * RANKS * FEAT)))
    W16 = _bf16(np.ascontiguousarray(np.asarray(W, dtype=np.float32)))
    b32 = np.asarray(b, dtype=np.float32)
    btile = np.tile(b32[None, :], (P, 1)).astype(np.float32)
    cmp_t = np.tile(np.arange(RW, dtype=np.float32)[None, :],
                    (P, 1)).astype(np.float16)
    ident = _bf16(np.eye(P, dtype=np.float32))

    in_maps = []
    for m in range(NCORES):
        sel = order_all[core_starts[m]:core_starts[m + 1]]
        midx, mrel, mnorm = idx_local[sel], rel[sel], norm[sel]

        rng = np.random.default_rng(12345 + m)
        idx16 = rng.integers(0, CHUNK_ROWS, slots_total).astype(np.int16)
        relq = np.full(slots_total, -1.0, dtype=np.float32)
        nrm = np.zeros(slots_total, dtype=np.float32)
        pos = 0
        for (c, w, blk0, nblk) in regions:
            cnt = int(counts[m, c, w])
            s0 = blk0 * P
            idx16[s0:s0 + cnt] = midx[pos:pos + cnt]
            relq[s0:s0 + cnt] = mrel[pos:pos + cnt]
            nrm[s0:s0 + cnt] = mnorm[pos:pos + cnt]
            pos += cnt
        assert pos == len(sel)

        idx_tile = np.zeros((P, slots_total // 16), dtype=np.int16)
        for (c, blk0, nblk) in calls:
            s0, s1 = blk0 * P, (blk0 + nblk) * P
            seg = idx16[s0:s1].reshape(-1, 16).T
            idx_tile[:, s0 // 16:s1 // 16] = np.tile(seg, (8, 1))

        # self-loop term pre-scaled on host, [feat, APAD dst] layout,
        # permuted to the balanced dst slot order
        xs = np.zeros((APAD, FEAT), dtype=np.float32)
        xs[newloc[m]] = (x32[m * NPC:(m + 1) * NPC]
                         * dinv2[m * NPC:(m + 1) * NPC, None])
        xslT = np.ascontiguousarray(xs.T)

        in_maps.append({
            "xgr": xgr,
            "idx": idx_tile,
            "rel": relq.reshape(nb_total, P).T.copy(),
            "nrm": nrm.reshape(nb_total, P).T.copy(),
            "cmp": cmp_t,
            "xslT": _bf16(xslT),
            "Wt": W16,
            "btile": btile,
            "ident": ident,
        })
    return meta, in_maps


def _build_kernel(meta, repeat=1, mode="full"):
    nb_total = meta["nb_total"]
    slots_total = meta["slots_total"]
    regions = meta["regions"]
    calls = meta["calls"]
    gdt = mybir.dt.bfloat16
    do_gather = mode in ("full", "gather_only")
    do_compute = mode in ("full", "compute_only")

    nc = bacc.Bacc("TRN2", target_bir_lowering=False, debug=False,
                   num_devices=NCORES)
    xgr = nc.dram_tensor("xgr", [P, NCHUNK * RANKS * FEAT], gdt,
                         kind="ExternalInput")
    idx = nc.dram_tensor("idx", [P, slots_total // 16], mybir.dt.int16,
                         kind="ExternalInput")
    rel = nc.dram_tensor("rel", [P, nb_total], mybir.dt.float32,
                         kind="ExternalInput")
    nrm = nc.dram_tensor("nrm", [P, nb_total], mybir.dt.float32,
                         kind="ExternalInput")
    cmp_d = nc.dram_tensor("cmp", [P, RW], mybir.dt.float16,
                           kind="ExternalInput")
    xslT = nc.dram_tensor("xslT", [FEAT, APAD], gdt, kind="ExternalInput")
    Wt = nc.dram_tensor("Wt", [FEAT, FEAT], gdt, kind="ExternalInput")
    btile = nc.dram_tensor("btile", [P, FEAT], mybir.dt.float32,
                           kind="ExternalInput")
    ident = nc.dram_tensor("ident", [P, P], gdt, kind="ExternalInput")
    out = nc.dram_tensor("out", [DPAD, FEAT], mybir.dt.float32,
                         kind="ExternalOutput")

    Bm = meta["B"]
    first_c = {}
    last_c = {}
    for w in range(NR):
        cs = [c for c in range(NCHUNK) if Bm[c, w] > 0]
        first_c[w] = cs[0] if cs else None
        last_c[w] = cs[-1] if cs else None

    with tile.TileContext(nc) as tc:
        with (
            tc.tile_pool(name="const", bufs=1) as constp,
            tc.tile_pool(name="agg", bufs=1) as aggp,
            tc.tile_pool(name="chunk", bufs=2) as chunkp,
            tc.tile_pool(name="gbuf", bufs=2) as gbufp,
            tc.tile_pool(name="gsc", bufs=2) as gscp,
            tc.tile_pool(name="sel", bufs=6) as selp,
            tc.tile_pool(name="xsl", bufs=3) as xslp,
            tc.tile_pool(name="ps1", bufs=2, space="PSUM") as ps1p,
            tc.tile_pool(name="tpg", bufs=3, space="PSUM") as tpgp,
            tc.tile_pool(name="ps2", bufs=2, space="PSUM") as ps2p,
            tc.tile_pool(name="outst", bufs=3) as outp,
        ):
            idx_sb = constp.tile([P, slots_total // 16], mybir.dt.int16)
            rel_sb = constp.tile([P, nb_total], mybir.dt.float32)
            nrm_sb = constp.tile([P, nb_total], mybir.dt.float32)
            cmp_sb = constp.tile([P, RW], mybir.dt.float16)
            W_sb = constp.tile([FEAT, FEAT], gdt)
            b_sb = constp.tile([P, FEAT], mybir.dt.float32)
            id_sb = constp.tile([P, P], gdt)
            agg = aggp.tile([FEAT, APAD], gdt)

            for _rep in range(repeat):
                nc.sync.dma_start(out=idx_sb[:], in_=idx[:])
                nc.sync.dma_start(out=rel_sb[:], in_=rel[:])
                nc.sync.dma_start(out=nrm_sb[:], in_=nrm[:])
                nc.sync.dma_start(out=cmp_sb[:], in_=cmp_d[:])
                nc.sync.dma_start(out=W_sb[:], in_=Wt[:])
                nc.sync.dma_start(out=b_sb[:], in_=btile[:])
                nc.sync.dma_start(out=id_sb[:], in_=ident[:])

                chunk_sb = {}

                def load_chunk(c):
                    t = chunkp.tile([P, RANKS * FEAT], gdt, tag="chunk")
                    nc.sync.dma_start(
                        out=t[:],
                        in_=xgr[:, c * RANKS * FEAT:(c + 1) * RANKS * FEAT])
                    chunk_sb[c] = t

                load_chunk(0)
                if NCHUNK > 1:
                    load_chunk(1)

                gtiles = {}
                issued = set()
                call_of_block = {}
                for ci, (c, blk0, nblk) in enumerate(calls):
                    for bb in range(blk0, blk0 + nblk):
                        call_of_block[bb] = ci

                def gather_call(ci):
                    c, blk0, nblk = calls[ci]
                    nidx = nblk * P
                    g = gbufp.tile([P, 1, CALL_MAX_BLOCKS * P], gdt, tag="g")
                    if not do_gather:
                        nc.vector.memset(g[:, :, :nidx], 0.25)
                    if do_gather:
                        nc.gpsimd.dma_gather(
                            g[:, :, :nidx],
                            chunk_sb[c][:],
                            idx_sb[:, blk0 * 8:(blk0 + nblk) * 8],
                            nidx, nidx, FEAT,
                            transpose=True, single_packet=False,
                            sbuf_tokens_per_rank=P,
                            sbuf_free_dim_per_rank=FEAT * 2,
                        )
                    if do_compute:
                        gsc_call = gscp.tile([P, CALL_MAX_BLOCKS, FEAT], gdt,
                                             tag="gsc")
                        TB = 4
                        for b0 in range(0, nblk, TB):
                            nb2 = min(TB, nblk - b0)
                            tpg = tpgp.tile([P, TB, P], gdt, tag="tpg")
                            for k in range(nb2):
                                bb = b0 + k
                                nc.tensor.transpose(
                                    tpg[:, k, :],
                                    g[:, 0, bb * P:(bb + 1) * P], id_sb[:])
                            nc.scalar.activation(
                                gsc_call[:, b0:b0 + nb2, :],
                                tpg[:, :nb2, :],
                                mybir.ActivationFunctionType.Copy)
                        gtiles[ci] = (gsc_call, blk0, nblk)
                    else:
                        gtiles[ci] = (None, blk0, nblk)

                def phase2(w):
                    ps2 = ps2p.tile([P, FEAT], mybir.dt.float32, tag="ps2")
                    nc.tensor.matmul(out=ps2[:],
                                     lhsT=agg[:, w * P:(w + 1) * P],
                                     rhs=W_sb[:], start=True, stop=True)
                    ot = outp.tile([P, FEAT], mybir.dt.float32, tag="ot")
                    nc.vector.tensor_add(out=ot[:], in0=ps2[:], in1=b_sb[:])
                    nc.vector.tensor_scalar_max(ot[:], ot[:], 0.0)
                    nc.sync.dma_start(out=out[w * P:(w + 1) * P, :], in_=ot[:])

                for (c, r2, blk0, nblk) in regions:
                    d0 = r2 * RW
                    wsl = agg[:, d0:d0 + RW]
                    if not do_compute:
                        for j, gb in enumerate(range(blk0, blk0 + nblk)):
                            ci = call_of_block[gb]
                            if ci not in issued:
                                gather_call(ci)
                                issued.add(ci)
                        if c + 2 < NCHUNK and r2 == NR - 1:
                            load_chunk(c + 2)
                        continue
                    if nblk == 0:
                        if first_c[r2] is None and c == 0:
                            xsl_t = xslp.tile([FEAT, RW], gdt, tag="xsl")
                            nc.sync.dma_start(out=xsl_t[:],
                                              in_=xslT[:, d0:d0 + RW])
                            nc.vector.tensor_copy(out=wsl, in_=xsl_t[:])
                            for s in range(S):
                                if r2 * S + s < NW:
                                    phase2(r2 * S + s)
                        continue
                    if c == first_c[r2]:
                        xsl_t = xslp.tile([FEAT, RW], gdt, tag="xsl")
                        nc.sync.dma_start(out=xsl_t[:],
                                          in_=xslT[:, d0:d0 + RW])
                    ps = ps1p.tile([P, RW], mybir.dt.float32, tag="ps1")
                    for j, gb in enumerate(range(blk0, blk0 + nblk)):
                        ci = call_of_block[gb]
                        if ci not in issued:
                            gather_call(ci)
                            issued.add(ci)
                        g, cblk0, cnblk = gtiles[ci]
                        col = gb - cblk0
                        st = selp.tile([P, RW], gdt, tag="sel")
                        nc.vector.tensor_scalar(
                            out=st[:], in0=cmp_sb[:],
                            scalar1=rel_sb[:, gb:gb + 1],
                            scalar2=nrm_sb[:, gb:gb + 1],
                            op0=mybir.AluOpType.is_equal,
                            op1=mybir.AluOpType.mult,
                        )
                        nc.tensor.matmul(
                            out=ps[:], lhsT=g[:, col, :], rhs=st[:],
                            start=(j == 0), stop=(j == nblk - 1),
                        )
                    if c == first_c[r2]:
                        nc.vector.tensor_tensor(
                            out=wsl, in0=ps[:], in1=xsl_t[:],
                            op=mybir.AluOpType.add)
                    else:
                        nc.vector.tensor_add(out=wsl, in0=wsl, in1=ps[:])
                    if c == last_c[r2]:
                        for s in range(S):
                            if r2 * S + s < NW:
                                phase2(r2 * S + s)
                    if c + 2 < NCHUNK and r2 == NR - 1:
                        load_chunk(c + 2)
    nc.compile()
    return nc


def kernel(x, edge_index, edge_weight, W, b):
    assert x.shape == (N_NODES, FEAT)
    meta, in_maps = _host_prep(x, edge_index, edge_weight, W, b)
    nc = _build_kernel(meta)
    res = bass_utils.run_bass_kernel_spmd(
        nc, in_maps, core_ids=list(range(NCORES)), trace=False)
    outs = [res.results[m]["out"][meta["newloc"][m]]
            for m in range(NCORES)]
    return np.ascontiguousarray(np.concatenate(outs, axis=0))
